# revision 11
# baseline (speedup 1.0000x reference)
"""Trainium2 Bass kernel for nn_EnhancedEdgeEmbedding (GNN edge MLP).

Strategy (8 NeuronCores, SPMD):
  - Edges sharded across cores; node tables + MLP weights replicated.
  - Node features cast to bf16 and held RESIDENT IN SBUF as two half-tables
    (int16 gather-index limit => split at 32768). Per-edge feature rows are
    fetched with dma_gather (SBUF source, transpose mode) which lands them
    FEATURE-MAJOR [128, n_edges] -- no on-chip transposes for the MLP chain.
  - Edges are class-sorted on host by (src-half, dst-half) so every gather
    call addresses a single half-table with plain rebased int16 indices.
    Per-class counts are padded so all 8 cores get identical schedules
    (same NEFF for all cores).
  - node_coords gathered per edge from a 256B-padded f32 HBM table
    (dma_gather non-transpose), rel = dst-src on DVE, transposed to
    feature-major with a cheap TensorE transpose; coords path is full f32.
  - MLP: h1 = silu(x@W1+b1) [272->256], h2 = silu(h1@W2+b2) [256->128],
    ef = h2@W3+b3. Feature-major matmuls, SiLU+bias fused on ACT (bias is
    per-partition in feature-major layout). ef produced edge-major by using
    h2 (feature-major) as the stationary operand -- output DMAs contiguously.
  - Output: one [ECP, 132] f32 tensor per core (128 features + 3 coords +
    1 pad); host un-permutes and splits.
"""

import numpy as np
import ml_dtypes

import concourse.bacc as bacc
import concourse.tile as tile
from concourse import mybir
import concourse.bass as bass
from concourse.bass_utils import run_bass_kernel_spmd
from concourse.masks import make_identity
from concourse.tile_rust import add_dep_helper

BF16 = ml_dtypes.bfloat16

# Problem constants (hardcoded per spec)
N, E = 50000, 800000
D, EA, H = 128, 16, 128
NCORES = 8
S = 32768                     # int16 half-table split
NPAD = 50176                  # 392 * 128
RANKS_A, RANKS_B = 256, 136   # half-table ranks (x128 tokens)
NB = NPAD - S                 # 17408 tokens in half B
CH = 1024                     # edges per gather chunk (multiple of 128)
DMA_SCRATCH = 16384           # SWDGE descriptor carveout bytes/partition
GRP = 512                     # edges per matmul group
FEAT_GATHER = True            # debug: replace feature gathers with memset
COORD_GATHER = True           # debug: replace coords gathers with memset
SERIALIZE_TG = True           # chain transpose-gathers to avoid xbar overlap

F32 = mybir.dt.float32
BF = mybir.dt.bfloat16
I16 = mybir.dt.int16


# ---------------------------------------------------------------- host prep

def _wrap_idx(v):
    """int16 values [n] -> dma_gather index layout [128, n/16]."""
    n = v.shape[0]
    iw = v.reshape(n // 16, 16).T            # [16, n/16]
    return np.tile(iw, (8, 1)).astype(np.int16)


def host_prep(node_features, node_coords, edge_index, edge_attr):
    src = np.asarray(edge_index[0], dtype=np.int64).astype(np.int32)
    dst = np.asarray(edge_index[1], dtype=np.int64).astype(np.int32)
    ea = np.asarray(edge_attr, dtype=np.float32)

    cls = (src >= S).astype(np.int32) * 2 + (dst >= S).astype(np.int32)
    perm = np.argsort(cls, kind="stable")
    counts = np.bincount(cls, minlength=4)
    padded = ((counts + (NCORES * 128 - 1)) // (NCORES * 128)) * (NCORES * 128)
    n_k = (padded // NCORES).astype(np.int64)          # per-core class counts
    ECP = int(n_k.sum())

    # schedule: per class, list of chunk sizes (each a multiple of 128)
    sched = []
    for k in range(4):
        nk = int(n_k[k])
        chunks = [CH] * (nk // CH)
        if nk % CH:
            chunks.append(nk % CH)
        sched.append(chunks)

    # padded, class-sorted global edge stream
    TOT = int(padded.sum())
    src_s = np.empty(TOT, np.int32)
    dst_s = np.empty(TOT, np.int32)
    ea_s = np.zeros((TOT, EA), np.float32)
    orig = np.full(TOT, -1, np.int64)
    off = 0
    cstart = 0
    for k in range(4):
        ids = perm[cstart:cstart + counts[k]]
        cstart += counts[k]
        seg = slice(off, off + counts[k])
        src_s[seg] = src[ids]
        dst_s[seg] = dst[ids]
        ea_s[seg] = ea[ids]
        orig[seg] = ids
        pad_seg = slice(off + counts[k], off + padded[k])
        src_s[pad_seg] = 0 if k < 2 else S
        dst_s[pad_seg] = 0 if (k % 2) == 0 else S
        off += padded[k]

    # node tables
    nf = np.zeros((NPAD, D), np.float32)
    nf[:N] = np.asarray(node_features, dtype=np.float32)
    nfb = np.ascontiguousarray(nf.astype(BF16))
    ctb = np.zeros((NPAD, 64), np.float32)
    ctb[:N, 0:3] = np.asarray(node_coords, dtype=np.float32)

    # per-core streams -> per-core idx/eat arrays
    in_maps = []
    core_orig = []
    offs_k = np.concatenate([[0], np.cumsum(padded)])
    for c in range(NCORES):
        segs = []
        for k in range(4):
            a = int(offs_k[k] + c * n_k[k])
            segs.append(slice(a, a + int(n_k[k])))
        src_c = np.concatenate([src_s[s] for s in segs])
        dst_c = np.concatenate([dst_s[s] for s in segs])
        ea_c = np.concatenate([ea_s[s] for s in segs])
        core_orig.append(np.concatenate([orig[s] for s in segs]))

        idx_blocks = []
        base = 0
        for k in range(4):
            sb = S * (k >> 1)
            db = S * (k & 1)
            for n in sched[k]:
                sv = (src_c[base:base + n] - sb).astype(np.int16)
                dv = (dst_c[base:base + n] - db).astype(np.int16)
                idx_blocks.append(_wrap_idx(sv))
                idx_blocks.append(_wrap_idx(dv))
                base += n
        idx_c = np.hstack(idx_blocks) if idx_blocks else np.zeros((128, 0), np.int16)
        eat_c = np.ascontiguousarray(ea_c.T).astype(BF16)   # [16, ECP]
        in_maps.append({"idx": idx_c, "eat": eat_c})

    return in_maps, sched, ECP, core_orig, (nfb, ctb)


def weight_inputs(W1, b1, W2, b2, W3, b3, Wc1, bc1, Wc2, bc2):
    W1 = np.asarray(W1, np.float32)
    W2 = np.asarray(W2, np.float32)
    W3 = np.asarray(W3, np.float32)
    w = {}
    for h in range(2):
        w[f"W1a{h}"] = W1[0:128, h * 128:(h + 1) * 128].astype(BF16)
        w[f"W1b{h}"] = W1[128:256, h * 128:(h + 1) * 128].astype(BF16)
        w[f"W1c{h}"] = W1[256:272, h * 128:(h + 1) * 128].astype(BF16)
    w["W2_0"] = W2[0:128, :].astype(BF16)
    w["W2_1"] = W2[128:256, :].astype(BF16)
    w["W3"] = W3.astype(BF16)
    w["Wc1"] = np.asarray(Wc1, np.float32)                       # [3, 128]
    Wc2p = np.zeros((128, 4), np.float32)
    Wc2p[:, 0:3] = np.asarray(Wc2, np.float32)
    w["Wc2p"] = Wc2p
    b1t = np.stack([np.asarray(b1, np.float32)[0:128],
                    np.asarray(b1, np.float32)[128:256]], axis=1)  # [128, 2]
    w["b1t"] = np.ascontiguousarray(b1t)
    w["b2t"] = np.asarray(b2, np.float32).reshape(128, 1)
    w["bc1t"] = np.asarray(bc1, np.float32).reshape(128, 1)
    w["b3rep"] = np.tile(np.asarray(b3, np.float32).reshape(1, 128), (128, 1))
    bc2rep = np.zeros((128, 4), np.float32)
    bc2rep[:, 0:3] = np.asarray(bc2, np.float32).reshape(1, 3)
    w["bc2rep"] = np.tile(bc2rep[0:1], (128, 1))
    return w


# ---------------------------------------------------------------- device build

def build(sched, ECP):
    nc = bacc.Bacc(dynamic_dma_scratch_size=DMA_SCRATCH)
    p = {}
    p["nfb"] = nc.declare_dram_parameter("nfb", [NPAD, D], BF, isOutput=False)
    p["ctb"] = nc.declare_dram_parameter("ctb", [NPAD, 64], F32, isOutput=False)
    p["idx"] = nc.declare_dram_parameter("idx", [128, ECP // 8], I16, isOutput=False)
    p["eat"] = nc.declare_dram_parameter("eat", [16, ECP], BF, isOutput=False)
    for nmame, shape, dt in [
        ("W1a0", [128, 128], BF), ("W1a1", [128, 128], BF),
        ("W1b0", [128, 128], BF), ("W1b1", [128, 128], BF),
        ("W1c0", [16, 128], BF), ("W1c1", [16, 128], BF),
        ("W2_0", [128, 128], BF), ("W2_1", [128, 128], BF),
        ("W3", [128, 128], BF),
        ("Wc1", [3, 128], F32), ("Wc2p", [128, 4], F32),
        ("b1t", [128, 2], F32), ("b2t", [128, 1], F32),
        ("bc1t", [128, 1], F32), ("b3rep", [128, 128], F32),
        ("bc2rep", [128, 4], F32),
    ]:
        p[nmame] = nc.declare_dram_parameter(nmame, shape, dt, isOutput=False)
    out = nc.declare_dram_parameter("out", [ECP, 132], F32, isOutput=True)

    with tile.TileContext(nc) as tc:
        with tc.tile_pool(name="const", bufs=1) as const, \
             tc.tile_pool(name="gath", bufs=2) as gath, \
             tc.tile_pool(name="ip", bufs=2) as ip, \
             tc.tile_pool(name="hp", bufs=2) as hp, \
             tc.tile_pool(name="ep", bufs=3) as epool, \
             tc.tile_pool(name="op", bufs=3) as op, \
             tc.tile_pool(name="psA", bufs=1, space="PSUM") as psA, \
             tc.tile_pool(name="psB", bufs=2, space="PSUM") as psB, \
             tc.tile_pool(name="psC", bufs=1, space="PSUM") as psC:


            # weights / consts
            wt = {}
            for nm in ["W1a0", "W1a1", "W1b0", "W1b1", "W2_0", "W2_1", "W3"]:
                wt[nm] = const.tile([128, 128], BF, tag=nm, name=nm)
                nc.sync.dma_start(out=wt[nm][:], in_=p[nm][:, :])
            for nm in ["W1c0", "W1c1"]:
                wt[nm] = const.tile([16, 128], BF, tag=nm, name=nm)
                nc.sync.dma_start(out=wt[nm][:], in_=p[nm][:, :])
            wc1 = const.tile([3, 128], F32)
            nc.sync.dma_start(out=wc1[:], in_=p["Wc1"][:, :])
            wc2p = const.tile([128, 4], F32)
            nc.sync.dma_start(out=wc2p[:], in_=p["Wc2p"][:, :])
            b1t = const.tile([128, 2], F32)
            nc.sync.dma_start(out=b1t[:], in_=p["b1t"][:, :])
            b2t = const.tile([128, 1], F32)
            nc.sync.dma_start(out=b2t[:], in_=p["b2t"][:, :])
            bc1t = const.tile([128, 1], F32)
            nc.sync.dma_start(out=bc1t[:], in_=p["bc1t"][:, :])
            b3rep = const.tile([128, 128], F32)
            nc.sync.dma_start(out=b3rep[:], in_=p["b3rep"][:, :])
            bc2rep = const.tile([128, 4], F32)
            nc.sync.dma_start(out=bc2rep[:], in_=p["bc2rep"][:, :])
            ident = const.tile([128, 128], F32)
            make_identity(nc, ident[:])
            identb = const.tile([128, 128], BF)
            make_identity(nc, identb[:])

            base = 0     # edge offset within core stream
            icoff = 0    # column offset in idx tensor
            prev_tg = None
            for k in range(4):
                tS_ap = p["nfb"][0:S, :] if (k >> 1) == 0 else p["nfb"][S:NPAD, :]
                tD_ap = p["nfb"][0:S, :] if (k & 1) == 0 else p["nfb"][S:NPAD, :]
                cS_ap = p["ctb"][0:S, :] if (k >> 1) == 0 else p["ctb"][S:NPAD, :]
                cD_ap = p["ctb"][0:S, :] if (k & 1) == 0 else p["ctb"][S:NPAD, :]
                for n in sched[k]:
                    nc16 = n // 16
                    itS = ip.tile([128, CH // 16], I16, tag="itS")
                    nc.sync.dma_start(out=itS[:, :nc16], in_=p["idx"][:, icoff:icoff + nc16])
                    itD = ip.tile([128, CH // 16], I16, tag="itD")
                    nc.sync.dma_start(out=itD[:, :nc16], in_=p["idx"][:, icoff + nc16:icoff + 2 * nc16])
                    icoff += 2 * nc16

                    gS = gath.tile([128, CH // 128, D], BF, tag="gS")
                    nc.gpsimd.dma_gather(
                        gS[:, :n // 128, :], tS_ap, itS[:, :nc16],
                        num_idxs=n, num_idxs_reg=n, elem_size=D, transpose=False)
                    gD = gath.tile([128, CH // 128, D], BF, tag="gD")
                    nc.gpsimd.dma_gather(
                        gD[:, :n // 128, :], tD_ap, itD[:, :nc16],
                        num_idxs=n, num_idxs_reg=n, elem_size=D, transpose=False)

                    cS = gath.tile([128, CH // 128, 64], F32, tag="cS")
                    cD = gath.tile([128, CH // 128, 64], F32, tag="cD")
                    if COORD_GATHER:
                        nc.gpsimd.dma_gather(
                            cS[:, :n // 128, :], cS_ap, itS[:, :nc16],
                            num_idxs=n, num_idxs_reg=n, elem_size=64, transpose=False)
                        nc.gpsimd.dma_gather(
                            cD[:, :n // 128, :], cD_ap, itD[:, :nc16],
                            num_idxs=n, num_idxs_reg=n, elem_size=64, transpose=False)
                    else:
                        nc.vector.memset(cS[:, :n // 128, :], 0.0)
                        nc.vector.memset(cD[:, :n // 128, :], 0.0)

                    for off in range(0, n, GRP):
                        gn = min(GRP, n - off)
                        ns = gn // 128
                        eatt = epool.tile([16, GRP], BF, tag="eatt")
                        nc.sync.dma_start(out=eatt[:, :gn],
                                          in_=p["eat"][:, base + off:base + off + gn])

                        # ---- transpose gathered subtiles to feature-major
                        xS = hp.tile([128, GRP], BF, tag="xS")
                        xD = hp.tile([128, GRP], BF, tag="xD")
                        for s2 in range(ns):
                            ptS = psB.tile([128, 128], BF, tag="pt")
                            nc.tensor.transpose(
                                ptS[:], gS[:, off // 128 + s2, :], identb[:])
                            nc.scalar.copy(out=xS[:, s2 * 128:(s2 + 1) * 128],
                                           in_=ptS[:])
                            ptD = psB.tile([128, 128], BF, tag="pt")
                            nc.tensor.transpose(
                                ptD[:], gD[:, off // 128 + s2, :], identb[:])
                            nc.vector.tensor_copy(
                                out=xD[:, s2 * 128:(s2 + 1) * 128], in_=ptD[:])

                        # ---- h1 (feature-major, 2 M-halves x 3 K-chunks)
                        h1s = []
                        for h in range(2):
                            ph = psA.tile([128, GRP], F32, tag=f"p1{h}")
                            nc.tensor.matmul(ph[:, :gn], wt[f"W1a{h}"][:],
                                             xS[:, :gn], start=True, stop=False)
                            nc.tensor.matmul(ph[:, :gn], wt[f"W1b{h}"][:],
                                             xD[:, :gn], start=False, stop=False)
                            nc.tensor.matmul(ph[:, :gn], wt[f"W1c{h}"][:],
                                             eatt[:, :gn], start=False, stop=True)
                            hs = hp.tile([128, GRP], BF, tag=f"h1s{h}")
                            nc.scalar.activation(out=hs[:, :gn], in_=ph[:, :gn],
                                                 func=mybir.ActivationFunctionType.Silu,
                                                 bias=b1t[:, h:h + 1], scale=1.0)
                            h1s.append(hs)

                        # ---- h2
                        p2 = psA.tile([128, GRP], F32, tag="p2")
                        nc.tensor.matmul(p2[:, :gn], wt["W2_0"][:], h1s[0][:, :gn],
                                         start=True, stop=False)
                        nc.tensor.matmul(p2[:, :gn], wt["W2_1"][:], h1s[1][:, :gn],
                                         start=False, stop=True)
                        h2s = hp.tile([128, GRP], BF, tag="h2s")
                        nc.scalar.activation(out=h2s[:, :gn], in_=p2[:, :gn],
                                             func=mybir.ActivationFunctionType.Silu,
                                             bias=b2t[:, 0:1], scale=1.0)

                        # ---- rel (edge-major) for this group
                        rel = hp.tile([128, GRP // 128, 3], F32, tag="rel")
                        nc.vector.tensor_tensor(
                            out=rel[:, :ns, :],
                            in0=cD[:, off // 128:off // 128 + ns, 0:3],
                            in1=cS[:, off // 128:off // 128 + ns, 0:3],
                            op=mybir.AluOpType.subtract)

                        out_t = op.tile([128, GRP // 128, 132], F32, tag="out_t")
                        for s in range(ns):
                            # ef = h2 @ W3 + b3 (edge-major out)
                            p3 = psB.tile([128, 128], F32, tag="pt")
                            nc.tensor.matmul(p3[:], h2s[:, s * 128:(s + 1) * 128],
                                             wt["W3"][:], start=True, stop=True)
                            nc.vector.tensor_tensor(out=out_t[:, s, 0:128], in0=p3[:],
                                                    in1=b3rep[:], op=mybir.AluOpType.add)

                            # coords: relT -> hc -> out
                            pT = psC.tile([3, 128], F32, tag="pT")
                            nc.tensor.transpose(pT[:], rel[:, s, :], ident[:])
                            relT = hp.tile([3, 128], F32, tag="relT")
                            nc.vector.tensor_copy(out=relT[:], in_=pT[:])
                            pc = psC.tile([128, 128], F32, tag="pc")
                            nc.tensor.matmul(pc[:], wc1[:], relT[:], start=True, stop=True)
                            hcs = hp.tile([128, 128], F32, tag="hcs")
                            nc.scalar.activation(out=hcs[:], in_=pc[:],
                                                 func=mybir.ActivationFunctionType.Silu,
                                                 bias=bc1t[:, 0:1], scale=1.0)
                            po = psC.tile([128, 4], F32, tag="po")
                            nc.tensor.matmul(po[:], hcs[:], wc2p[:], start=True, stop=True)
                            nc.vector.tensor_tensor(out=out_t[:, s, 128:132], in0=po[:],
                                                    in1=bc2rep[:], op=mybir.AluOpType.add)

                        dst_ap = out[base + off:base + off + gn, :].rearrange(
                            "(s q) c -> q s c", q=128)
                        nc.sync.dma_start(out=dst_ap, in_=out_t[:, :ns, :])
                    base += n

    nc.finalize()
    return nc


# ---------------------------------------------------------------- entry point

_CACHE = {}


def kernel(node_features, node_coords, edge_index, edge_attr,
           W1, b1, W2, b2, W3, b3, Wc1, bc1, Wc2, bc2):
    in_maps, sched, ECP, core_orig, (nfb, ctb) = host_prep(
        node_features, node_coords, edge_index, edge_attr)
    wts = weight_inputs(W1, b1, W2, b2, W3, b3, Wc1, bc1, Wc2, bc2)

    key = tuple(tuple(s) for s in sched)
    if key not in _CACHE:
        _CACHE[key] = build(sched, ECP)
    nc = _CACHE[key]

    for m in in_maps:
        m.update(wts)
        m["nfb"] = nfb
        m["ctb"] = ctb

    res = run_bass_kernel_spmd(nc, in_maps, core_ids=list(range(NCORES)))

    ef = np.empty((E, 128), np.float32)
    ec = np.empty((E, 3), np.float32)
    for c in range(NCORES):
        o = res.results[c]["out"]
        orig = core_orig[c]
        v = orig >= 0
        ef[orig[v]] = o[v, 0:128]
        ec[orig[v]] = o[v, 128:131]
    return ef, ec
